# revision 11
# baseline (speedup 1.0000x reference)
"""Dynamic filter layer on 8 trn2 NeuronCores — v11 (patch-matmul, blocked x).

out[b,i,j,c] = sum_{di,dj} x[b,i+di,j+dj,c] * flow[b,i,j,di*K+dj]

B=8, H=W=256, C=64, K=5, Ho=Wo=252. Data-parallel over batch, one
sample per core (SPMD, no collectives).

Same matmul structure as v9 (one k=128 MM per 4x12-pixel patch:
stationary lhsT = 8x16 x-window [128, c=64], moving rhs = host-packed
"staircase" flow [128, 48], psum out [c, 48]), but x ships as
NON-overlapping 4-row blocks (parity-packed: block b at partition half
(b%2)*64) -- 12.1 MB/core instead of 21.7 -- and the 2x row overlap of
consecutive windows is reconstructed on-chip:

  window lp (patch row) = blocks (lp, lp+1).
  even lp: both halves already sit at the right partition halves of
           one shipped slot (lhsT read in place, zero copies).
  odd lp:  2 half-copies on DVE/ACT (engines support a partition-base
           offset between src and dst) into a working tile.

Measured on HW: LDW+MM pairs with fresh weights run ~49 ns (LDW
overlaps MM via the background weight buffer; identical consecutive
weights are deduped to ~20 ns/MM) -> PE ~65 us, hidden under DMA.
DMA/core: x 12.1 + staircase 16.3 + out 8.1 = 36.5 MB.

NOTE (hard-won): a PSUM bank must never be written by PE matmuls from
different row-group halves in close succession -- concurrent sub-array
drains to one bank abort the NEFF. All v11 matmuls are full k=128
(single row group), which sidesteps this entirely.
"""

import numpy as np

H = 256
W = 256
C = 64
K = 5
HO = H - K + 1  # 252
WO = W - K + 1  # 252
NCORES = 8

R = 4  # output rows per patch
JT = 12  # output cols per patch
PA = R + K - 1  # 8 input rows per patch window
PJ = JT + K - 1  # 16 input cols per patch window
NPI = HO // R  # 63 row-patches
NPJ = WO // JT  # 21 col-patches
NB = H // R  # 64 x row-blocks (4 rows each, no overlap)
SUP = 9  # row-patches per super-row (one input DMA each)
NSUP = NPI // SUP  # 7
BSLOT = (SUP + 1) // 2  # 5 shipped block-slots per super (2 halves each)
NODD = SUP // 2  # 4 odd windows per super (need copies)
GPR = 3  # psum banks per row-patch
PPG = NPJ // GPR  # 7 patch slots per bank
GF = PPG * R * JT  # 336 f32 per bank
NOUT = (NSUP + 1) // 2  # 4 output super-pair slots

_nc_cache = {}
ABLATE = "none"  # none|noevac|dmaonly


def _build(reps=1):
    """reps>1 wraps the whole body in a HW loop (timing calibration only)."""
    global _nc_cache
    key = (reps, ABLATE)
    if key in _nc_cache:
        return _nc_cache[key]

    import contextlib

    import concourse.bacc as bacc
    import concourse.tile as tile
    from concourse import mybir

    f32 = mybir.dt.float32
    bf16 = mybir.dt.bfloat16

    nc = bacc.Bacc(None, target_bir_lowering=False)
    xbd = nc.dram_tensor(
        "xbd", [NSUP, 128, BSLOT, NPJ, C], bf16, kind="ExternalInput"
    )
    std = nc.dram_tensor(
        "std", [NSUP, 128, SUP, NPJ, R * JT], bf16, kind="ExternalInput"
    )
    outd = nc.dram_tensor(
        "outd", [NOUT, 128, SUP, GPR, GF], bf16, kind="ExternalOutput"
    )

    with tile.TileContext(nc) as tc:
        with (
            tc.tile_pool(name="xp", bufs=3) as xp,
            tc.tile_pool(name="wp", bufs=2) as wp,
            tc.tile_pool(name="fp", bufs=3) as fp,
            tc.tile_pool(name="op", bufs=2) as op,
            tc.tile_pool(name="pp", bufs=2, space="PSUM") as pp,
        ):
            with tc.For_i(0, reps, 1) if reps > 1 else contextlib.nullcontext():
                stage = None
                for s in range(NSUP):
                    xt = xp.tile([128, BSLOT, NPJ, C], bf16, tag="x")
                    nc.sync.dma_start(out=xt, in_=xbd[s, :, :, :, :])
                    st = fp.tile([128, SUP, NPJ, R * JT], bf16, tag="f")
                    nc.sync.dma_start(out=st, in_=std[s, :, :, :, :])
                    # Odd windows lp=2t+1 (t<NODD): low half = block 2t+1
                    # (at xt[64:128, t]), high half = block 2t+2 (at
                    # xt[0:64, t+1]). Copy both into xw slot t.
                    xw = wp.tile([128, NODD, NPJ, C], bf16, tag="xw")
                    for t in range(NODD):
                        if (s + t) % 2 == 0:
                            nc.vector.tensor_copy(
                                xw[0:64, t, :, :], xt[64:128, t, :, :]
                            )
                            nc.scalar.copy(
                                out=xw[64:128, t, :, :],
                                in_=xt[0:64, t + 1, :, :],
                            )
                        else:
                            nc.scalar.copy(
                                out=xw[0:64, t, :, :],
                                in_=xt[64:128, t, :, :],
                            )
                            nc.vector.tensor_copy(
                                xw[64:128, t, :, :], xt[0:64, t + 1, :, :]
                            )
                    if s % 2 == 0:
                        stage = op.tile(
                            [128, SUP, GPR, GF], bf16, tag="stage"
                        )
                    pb = 64 * (s % 2)
                    for lp in range(SUP):
                        ps = pp.tile([C, GPR, 512], f32, tag="ps")
                        if ABLATE in ("noevac", "dmaonly"):
                            nc.vector.memset(ps[:, :, :1], 0.0)
                        if ABLATE != "dmaonly":
                            for pa in range(NPJ):
                                g = pa % GPR
                                off = 48 * (pa // GPR)
                                if lp % 2 == 0:
                                    lhsT = xt[:, lp // 2, pa, :]
                                else:
                                    lhsT = xw[:, lp // 2, pa, :]
                                nc.tensor.matmul(
                                    ps[:, g, off : off + 48],
                                    lhsT,
                                    st[:, lp, pa, :],
                                    start=True,
                                    stop=True,
                                )
                        if ABLATE == "none":
                            dst = stage[pb : pb + 64, lp, :, :]
                            src = ps[:, :, :GF]
                            if (s * SUP + lp) % 2 == 0:
                                nc.vector.tensor_copy(dst, src)
                            else:
                                nc.scalar.copy(out=dst, in_=src)
                        else:
                            nc.vector.memset(
                                stage[pb : pb + 64, lp, :1, :1], 0.0
                            )
                    if s % 2 == 1:
                        nc.sync.dma_start(
                            out=outd[s // 2, :, :, :, :], in_=stage
                        )
                    elif s == NSUP - 1:
                        nc.sync.dma_start(
                            out=outd[s // 2, :64, :, :, :],
                            in_=stage[:64, :, :, :],
                        )

    nc.finalize()
    _nc_cache[key] = nc
    return nc


def _bf16():
    import ml_dtypes

    return ml_dtypes.bfloat16


def _pack_x(x_core):
    """f32 [H,W,C] -> xbd bf16 [NSUP, 128, BSLOT, NPJ, C]:
    block b (x rows 4b..4b+3) of super s (bb = b - SUP*s in [0, SUP+1))
    sits at partition half 64*(bb%2), slot bb//2:
    xbd[s, 64*(bb%2) + a*PJ + jp, bb//2, pj, c] = x[4b+a, JT*pj+jp, c]."""
    bf16 = _bf16()
    xb = np.ascontiguousarray(np.asarray(x_core).astype(bf16))
    s0, s1, s2 = xb.strides
    win = np.lib.stride_tricks.as_strided(
        xb,
        shape=(NB, R, NPJ, PJ, C),
        strides=(R * s0, s0, JT * s1, s1, s2),
    )
    # [b, a, pj, jp, c] -> [b, (a,jp)=64, pj, c]
    xw = win.transpose(0, 1, 3, 2, 4).reshape(NB, 64, NPJ, C)
    xbd = np.zeros((NSUP, 128, BSLOT, NPJ, C), dtype=bf16)
    for s in range(NSUP):
        for bb in range(SUP + 1):
            par = 64 * (bb % 2)
            xbd[s, par : par + 64, bb // 2] = xw[SUP * s + bb]
    return np.ascontiguousarray(xbd)


def _pack_flow(flow_core):
    """f32 [HO,WO,K*K] -> std bf16 [NSUP, 128, SUP, NPJ, R*JT].
    Staircase per patch pi = SUP*s + lp, pj:
      std[s, (r+di)*PJ + (jo+dj), lp, pj, r*JT+jo]
          = flow[R*pi+r, JT*pj+jo, di*K+dj], zero elsewhere."""
    bf16 = _bf16()
    fb = np.ascontiguousarray(np.asarray(flow_core).astype(bf16))
    s0, s1, s2 = fb.strides
    fw = np.lib.stride_tricks.as_strided(
        fb,
        shape=(NPI, R, NPJ, JT, K * K),
        strides=(R * s0, s0, JT * s1, s1, s2),
    )
    # fw[pi, r, pj, jo, t]
    strp = np.zeros((NPI, 128, NPJ, R * JT), dtype=bf16)
    rr, jj = np.meshgrid(np.arange(R), np.arange(JT), indexing="ij")
    col = (rr * JT + jj).ravel()  # (48,)
    for t in range(K * K):
        di, dj = divmod(t, K)
        pidx = ((rr + di) * PJ + (jj + dj)).ravel()  # (48,)
        src = fw[:, :, :, :, t].transpose(0, 1, 3, 2).reshape(NPI, R * JT, NPJ)
        strp[:, pidx, :, col] = src.transpose(1, 0, 2)
    std = (
        strp.reshape(NSUP, SUP, 128, NPJ, R * JT)
        .transpose(0, 2, 1, 3, 4)
    )
    return np.ascontiguousarray(std)


def _core_inputs(x_core, flow_core):
    return {"xbd": _pack_x(x_core), "std": _pack_flow(flow_core)}


def _postprocess_core(outd_core):
    """bf16 [NOUT, 128, SUP, GPR, GF] -> f32 [HO, WO, C].
    Patch pa sits at psum bank g=pa%GPR, slot pa//GPR."""
    o = np.asarray(outd_core, dtype=np.float32)
    # [q, (par, c), ro, g, (slot, r, jo)]
    o = o.reshape(NOUT, 2, C, SUP, GPR, PPG, R, JT)
    # -> [q, par, ro, r, slot, g, jo, c]; pj = GPR*slot + g
    o = o.transpose(0, 1, 3, 6, 5, 4, 7, 2)
    o = o.reshape(NOUT * 2, SUP, R, GPR * PPG, JT, C)[:NSUP]
    return np.ascontiguousarray(o.reshape(HO, WO, C))


def _run(x, flow, trace=False):
    """x: [8,H,W,C] f32, flow: [8,HO,WO,25] f32 -> (out [8,HO,WO,C], res)"""
    from concourse.bass_utils import run_bass_kernel_spmd

    nc = _build()
    in_maps = [_core_inputs(x[b], flow[b]) for b in range(NCORES)]
    res = run_bass_kernel_spmd(
        nc, in_maps, core_ids=list(range(NCORES)), trace=trace
    )
    out = np.stack(
        [_postprocess_core(r["outd"]) for r in res.results], axis=0
    )
    return out, res


def kernel(x, flow, ksize=None, **_unused):
    x = np.asarray(x, dtype=np.float32)
    flow = np.asarray(flow, dtype=np.float32)
    out, _ = _run(x, flow, trace=False)
    return out


# revision 12
# speedup vs baseline: 1.3302x; 1.3302x over previous
"""Dynamic filter layer on 8 trn2 NeuronCores — v11 (patch-matmul, blocked x).

out[b,i,j,c] = sum_{di,dj} x[b,i+di,j+dj,c] * flow[b,i,j,di*K+dj]

B=8, H=W=256, C=64, K=5, Ho=Wo=252. Data-parallel over batch, one
sample per core (SPMD, no collectives).

Same matmul structure as v9 (one k=128 MM per 4x12-pixel patch:
stationary lhsT = 8x16 x-window [128, c=64], moving rhs = host-packed
"staircase" flow [128, 48], psum out [c, 48]), but x ships as
NON-overlapping 4-row blocks (parity-packed: block b at partition half
(b%2)*64) -- 12.1 MB/core instead of 21.7 -- and the 2x row overlap of
consecutive windows is reconstructed on-chip:

  window lp (patch row) = blocks (lp, lp+1).
  even lp: both halves already sit at the right partition halves of
           one shipped slot (lhsT read in place, zero copies).
  odd lp:  2 half-copies on DVE/ACT (engines support a partition-base
           offset between src and dst) into a working tile.

Measured on HW: LDW+MM pairs with fresh weights run ~49 ns (LDW
overlaps MM via the background weight buffer; identical consecutive
weights are deduped to ~20 ns/MM) -> PE ~65 us, hidden under DMA.
DMA/core: x 12.1 + staircase 16.3 + out 8.1 = 36.5 MB.

NOTE (hard-won): a PSUM bank must never be written by PE matmuls from
different row-group halves in close succession -- concurrent sub-array
drains to one bank abort the NEFF. All v11 matmuls are full k=128
(single row group), which sidesteps this entirely.
"""

import numpy as np

H = 256
W = 256
C = 64
K = 5
HO = H - K + 1  # 252
WO = W - K + 1  # 252
NCORES = 8

R = 4  # output rows per patch
JT = 12  # output cols per patch
PA = R + K - 1  # 8 input rows per patch window
PJ = JT + K - 1  # 16 input cols per patch window
NPI = HO // R  # 63 row-patches
NPJ = WO // JT  # 21 col-patches
NB = H // R  # 64 x row-blocks (4 rows each, no overlap)
SUP = 9  # row-patches per super-row (one input DMA each)
NSUP = NPI // SUP  # 7
BSLOT = (SUP + 1) // 2  # 5 shipped block-slots per super (2 halves each)
NODD = SUP // 2  # 4 odd windows per super (need copies)
GPR = 3  # psum banks per row-patch
PPG = NPJ // GPR  # 7 patch slots per bank
GF = PPG * R * JT  # 336 f32 per bank
NOUT = (NSUP + 1) // 2  # 4 output super-pair slots

_nc_cache = {}
ABLATE = "none"  # none|noevac|dmaonly


def _build(reps=1):
    """reps>1 wraps the whole body in a HW loop (timing calibration only)."""
    global _nc_cache
    key = (reps, ABLATE)
    if key in _nc_cache:
        return _nc_cache[key]

    import contextlib

    import concourse.bacc as bacc
    import concourse.tile as tile
    from concourse import mybir

    f32 = mybir.dt.float32
    bf16 = mybir.dt.bfloat16

    nc = bacc.Bacc(None, target_bir_lowering=False)
    xbd = nc.dram_tensor(
        "xbd", [NSUP, 128, BSLOT, NPJ, C], bf16, kind="ExternalInput"
    )
    std = nc.dram_tensor(
        "std", [NSUP, 128, SUP, NPJ, R * JT], bf16, kind="ExternalInput"
    )
    outd = nc.dram_tensor(
        "outd", [NOUT, 128, SUP, GPR, GF], bf16, kind="ExternalOutput"
    )

    with tile.TileContext(nc) as tc:
        with (
            tc.tile_pool(name="xp", bufs=3) as xp,
            tc.tile_pool(name="wp", bufs=2) as wp,
            tc.tile_pool(name="fp", bufs=3) as fp,
            tc.tile_pool(name="op", bufs=2) as op,
            tc.tile_pool(name="pp", bufs=2, space="PSUM") as pp,
        ):
            with tc.For_i(0, reps, 1) if reps > 1 else contextlib.nullcontext():
                stage = None
                for s in range(NSUP):
                    xt = xp.tile([128, BSLOT, NPJ, C], bf16, tag="x")
                    nc.sync.dma_start(out=xt, in_=xbd[s, :, :, :, :])
                    st = fp.tile([128, SUP, NPJ, R * JT], bf16, tag="f")
                    nc.sync.dma_start(out=st, in_=std[s, :, :, :, :])
                    # Odd windows lp=2t+1 (t<NODD): low half = block 2t+1
                    # (at xt[64:128, t]), high half = block 2t+2 (at
                    # xt[0:64, t+1]). Copy both into xw slot t.
                    xw = wp.tile([128, NODD, NPJ, C], bf16, tag="xw")
                    for t in range(NODD):
                        if (s + t) % 2 == 0:
                            nc.vector.tensor_copy(
                                xw[0:64, t, :, :], xt[64:128, t, :, :]
                            )
                            nc.scalar.copy(
                                out=xw[64:128, t, :, :],
                                in_=xt[0:64, t + 1, :, :],
                            )
                        else:
                            nc.scalar.copy(
                                out=xw[0:64, t, :, :],
                                in_=xt[64:128, t, :, :],
                            )
                            nc.vector.tensor_copy(
                                xw[64:128, t, :, :], xt[0:64, t + 1, :, :]
                            )
                    if s % 2 == 0:
                        stage = op.tile(
                            [128, SUP, GPR, GF], bf16, tag="stage"
                        )
                    pb = 64 * (s % 2)
                    for lp in range(SUP):
                        # one single-bank psum tile per (row, g): finer
                        # pipelining -- each bank evacuates and frees
                        # independently instead of per-row 3-bank blocks
                        pss = []
                        for g in range(GPR):
                            ps = pp.tile(
                                [C, 512], f32, name="ps", tag="ps", bufs=7
                            )
                            pss.append(ps)
                            if ABLATE in ("noevac", "dmaonly"):
                                nc.vector.memset(ps[:, :1], 0.0)
                        if ABLATE != "dmaonly":
                            for pa in range(NPJ):
                                g = pa % GPR
                                off = 48 * (pa // GPR)
                                if lp % 2 == 0:
                                    lhsT = xt[:, lp // 2, pa, :]
                                else:
                                    lhsT = xw[:, lp // 2, pa, :]
                                nc.tensor.matmul(
                                    pss[g][:, off : off + 48],
                                    lhsT,
                                    st[:, lp, pa, :],
                                    start=True,
                                    stop=True,
                                )
                        if ABLATE == "none":
                            for g in range(GPR):
                                dst = stage[pb : pb + 64, lp, g, :]
                                src = pss[g][:, :GF]
                                if (s * SUP + lp + g) % 2 == 0:
                                    nc.vector.tensor_copy(dst, src)
                                else:
                                    nc.scalar.copy(out=dst, in_=src)
                        else:
                            nc.vector.memset(
                                stage[pb : pb + 64, lp, :1, :1], 0.0
                            )
                    if s % 2 == 1:
                        nc.sync.dma_start(
                            out=outd[s // 2, :, :, :, :], in_=stage
                        )
                    elif s == NSUP - 1:
                        nc.sync.dma_start(
                            out=outd[s // 2, :64, :, :, :],
                            in_=stage[:64, :, :, :],
                        )

    nc.finalize()
    _nc_cache[key] = nc
    return nc


def _bf16():
    import ml_dtypes

    return ml_dtypes.bfloat16


def _pack_x(x_core):
    """f32 [H,W,C] -> xbd bf16 [NSUP, 128, BSLOT, NPJ, C]:
    block b (x rows 4b..4b+3) of super s (bb = b - SUP*s in [0, SUP+1))
    sits at partition half 64*(bb%2), slot bb//2:
    xbd[s, 64*(bb%2) + a*PJ + jp, bb//2, pj, c] = x[4b+a, JT*pj+jp, c]."""
    bf16 = _bf16()
    xb = np.ascontiguousarray(np.asarray(x_core).astype(bf16))
    s0, s1, s2 = xb.strides
    win = np.lib.stride_tricks.as_strided(
        xb,
        shape=(NB, R, NPJ, PJ, C),
        strides=(R * s0, s0, JT * s1, s1, s2),
    )
    # [b, a, pj, jp, c] -> [b, (a,jp)=64, pj, c]
    xw = win.transpose(0, 1, 3, 2, 4).reshape(NB, 64, NPJ, C)
    xbd = np.zeros((NSUP, 128, BSLOT, NPJ, C), dtype=bf16)
    for s in range(NSUP):
        for bb in range(SUP + 1):
            par = 64 * (bb % 2)
            xbd[s, par : par + 64, bb // 2] = xw[SUP * s + bb]
    return np.ascontiguousarray(xbd)


def _pack_flow(flow_core):
    """f32 [HO,WO,K*K] -> std bf16 [NSUP, 128, SUP, NPJ, R*JT].
    Staircase per patch pi = SUP*s + lp, pj:
      std[s, (r+di)*PJ + (jo+dj), lp, pj, r*JT+jo]
          = flow[R*pi+r, JT*pj+jo, di*K+dj], zero elsewhere."""
    bf16 = _bf16()
    fb = np.ascontiguousarray(np.asarray(flow_core).astype(bf16))
    s0, s1, s2 = fb.strides
    fw = np.lib.stride_tricks.as_strided(
        fb,
        shape=(NPI, R, NPJ, JT, K * K),
        strides=(R * s0, s0, JT * s1, s1, s2),
    )
    # fw[pi, r, pj, jo, t]
    strp = np.zeros((NPI, 128, NPJ, R * JT), dtype=bf16)
    rr, jj = np.meshgrid(np.arange(R), np.arange(JT), indexing="ij")
    col = (rr * JT + jj).ravel()  # (48,)
    for t in range(K * K):
        di, dj = divmod(t, K)
        pidx = ((rr + di) * PJ + (jj + dj)).ravel()  # (48,)
        src = fw[:, :, :, :, t].transpose(0, 1, 3, 2).reshape(NPI, R * JT, NPJ)
        strp[:, pidx, :, col] = src.transpose(1, 0, 2)
    std = (
        strp.reshape(NSUP, SUP, 128, NPJ, R * JT)
        .transpose(0, 2, 1, 3, 4)
    )
    return np.ascontiguousarray(std)


def _core_inputs(x_core, flow_core):
    return {"xbd": _pack_x(x_core), "std": _pack_flow(flow_core)}


def _postprocess_core(outd_core):
    """bf16 [NOUT, 128, SUP, GPR, GF] -> f32 [HO, WO, C].
    Patch pa sits at psum bank g=pa%GPR, slot pa//GPR."""
    o = np.asarray(outd_core, dtype=np.float32)
    # [q, (par, c), ro, g, (slot, r, jo)]
    o = o.reshape(NOUT, 2, C, SUP, GPR, PPG, R, JT)
    # -> [q, par, ro, r, slot, g, jo, c]; pj = GPR*slot + g
    o = o.transpose(0, 1, 3, 6, 5, 4, 7, 2)
    o = o.reshape(NOUT * 2, SUP, R, GPR * PPG, JT, C)[:NSUP]
    return np.ascontiguousarray(o.reshape(HO, WO, C))


def _run(x, flow, trace=False):
    """x: [8,H,W,C] f32, flow: [8,HO,WO,25] f32 -> (out [8,HO,WO,C], res)"""
    from concourse.bass_utils import run_bass_kernel_spmd

    nc = _build()
    in_maps = [_core_inputs(x[b], flow[b]) for b in range(NCORES)]
    res = run_bass_kernel_spmd(
        nc, in_maps, core_ids=list(range(NCORES)), trace=trace
    )
    out = np.stack(
        [_postprocess_core(r["outd"]) for r in res.results], axis=0
    )
    return out, res


def kernel(x, flow, ksize=None, **_unused):
    x = np.asarray(x, dtype=np.float32)
    flow = np.asarray(flow, dtype=np.float32)
    out, _ = _run(x, flow, trace=False)
    return out
